# revision 32
# baseline (speedup 1.0000x reference)
"""GPT causal attention (B=2,S=4096,D=768,H=12,HD=64) on 8 NeuronCores.

Sharding: core c handles batch b=c//4 and head-group hg=c%4 (3 heads each);
the host sums the 4 head-group partials per batch and adds bo.

Per-core kernel (fp32 PSUM; fp8 scores, bf16 elsewhere, f16 output):
  - Q/K kept in fp8e4m3 DoubleRow layout [32, 2, S] per head (hd=32*slot+p),
    built by pair-of-qsb partition-shift DMAs from fp8 projection drains;
    the scores matmul runs in DoubleRow perf mode (0.5 cycles/row) into a
    flat 3-head fused PSUM tile [128, 1536]
  - softmax exp SPLIT across engines: Act runs exact exp on the first
    EXP_ACT_COLS flat columns per key chunk, DVE runs a Schraudolph
    bit-trick exp (i16 = s*0.125*log2e*128 + C, bitcast bf16) on the rest;
    the causal mask multiply runs on GpSimd (SBUF-only; GpSimd cannot
    read PSUM so it cannot share the exp work)
  - P@V: es is the STATIONARY operand, V [128,65] the moving one; col 64
    of V is ones so ctx PSUM col 64 accumulates the softmax denominator
  - normalization batched per window: 2 strided reciprocals + 4 broadcast
    tensor_tensor multiplies drain all 12 ctx qsub slots at once
  - ctx transposed to head-dim-major via PE pair-transposes; output
    projection accumulates all 4 q-blocks in one full PSUM bank (single
    drain instr), staged f16 and stored f16
  - PSUM: scores double-buffer 6 banks + 2 ctx banks. The QKV / output
    projections run in a pre-PV window (P@V deferred, es tiles buffered)
    REUSING the ctx banks as scratch; successive scratch items PING-PONG
    between the two banks so PE never waits for a drain to finish before
    starting the next item's matmuls (this was worth ~25us).
"""
import contextlib
import sys

sys.path.insert(0, "/opt/trn_rl_repo")

import numpy as np
import ml_dtypes

import concourse.bass as bass
import concourse.tile as tile
from concourse import bacc, mybir
from concourse.bass_utils import run_bass_kernel_spmd

B, S, D, H, HD = 2, 4096, 768, 12, 64
N_CORES = 8
HPC = 3           # heads per core
DH = HPC * HD     # 192 per-core head dims
KD = D // 128     # 6 contraction chunks
QSB = 512         # query superblock width
NQSB = S // QSB   # 8
NKC = S // 128    # 32 key chunks

f32 = mybir.dt.float32
f16 = mybir.dt.float16
bf16 = mybir.dt.bfloat16
i16 = mybir.dt.int16
fp8 = mybir.dt.float8e4
BF = ml_dtypes.bfloat16
EXP = mybir.ActivationFunctionType.Exp
MUL = mybir.AluOpType.mult
ADD = mybir.AluOpType.add
DR = mybir.MatmulPerfMode.DoubleRow

# Schraudolph exp constants (bf16 bit domain, folds the 1/sqrt(HD) scale)
L2E = 1.4426950408889634
SCH_A = 0.125 * L2E * 128.0
SCH_C = 4.43
SCH_B = 16256.0 - SCH_C

# flat columns (of 3*512 per key chunk) handled by Act's exact exp; the
# remainder goes to DVE's Schraudolph
EXP_ACT_COLS = 992
# outproj drain dout-chunks handled by Act (rest DVE)
OUTPROJ_ACT = (0, 3)
# engine for the QKV projection drains: "A" or "D" per pass index
QK_DRAIN_ENG = ("D", "D", "D")
V_COPY_ENG = "D"

_CACHE = {}


def build():
    nc = bacc.Bacc("TRN2", target_bir_lowering=False, debug=False,
                   num_devices=N_CORES)
    xT = nc.dram_tensor("xT", [D, S], bf16, kind="ExternalInput").ap()
    wqk = nc.dram_tensor("wqk", [D, HPC, 128], bf16, kind="ExternalInput").ap()
    wv = nc.dram_tensor("wv", [D, DH], bf16, kind="ExternalInput").ap()
    woT = nc.dram_tensor("woT", [DH, D], bf16, kind="ExternalInput").ap()
    qkb = nc.dram_tensor("qkb", [128, HPC], f32, kind="ExternalInput").ap()
    bvv = nc.dram_tensor("bvv", [1, DH], bf16, kind="ExternalInput").ap()
    tmk = nc.dram_tensor("tmk", [128, 128], bf16, kind="ExternalInput").ap()
    idm = nc.dram_tensor("idm", [128, 128], bf16, kind="ExternalInput").ap()
    outT = nc.dram_tensor("outT", [D, S], f16, kind="ExternalOutput").ap()

    with tile.TileContext(nc) as tc:
        with contextlib.ExitStack() as ctx:
            sb = ctx.enter_context(tc.tile_pool(name="sb", bufs=1))
            # ---- resident inputs ----
            wqk_sb = sb.tile([128, KD, HPC, 128], bf16, tag="wqk", name="wqk_sb")
            nc.sync.dma_start(
                out=wqk_sb, in_=wqk.rearrange("(ko p) h m -> p ko h m", p=128))
            xt = [sb.tile([128, S], bf16, tag=f"xt{k}", name=f"xt{k}")
                  for k in range(KD)]
            for k in range(KD):
                nc.sync.dma_start(out=xt[k][:, 0:QSB],
                                  in_=xT[k * 128:(k + 1) * 128, 0:QSB])
            qkb_sb = sb.tile([128, HPC], f32, tag="qkb", name="qkb_sb")
            nc.sync.dma_start(out=qkb_sb, in_=qkb)
            wv_sb = sb.tile([128, KD, DH], bf16, tag="wv", name="wv_sb")
            nc.sync.dma_start(
                out=wv_sb, in_=wv.rearrange("(ko p) m -> p ko m", p=128))
            bv_sb = sb.tile([1, DH], bf16, tag="bv", name="bv_sb")
            nc.sync.dma_start(out=bv_sb, in_=bvv)
            mask_sb = sb.tile([128, 128], bf16, tag="mk", name="mask_sb")
            nc.sync.dma_start(out=mask_sb, in_=tmk)
            id_sb = sb.tile([128, 128], bf16, tag="id", name="id_sb")
            nc.sync.dma_start(out=id_sb, in_=idm)
            wo_a = sb.tile([128, D], bf16, tag="woa", name="wo_a")
            # head-2 rows of woT duplicated into BOTH partition halves, so
            # either half of a pair-transposed cnb tile can contract with it
            wo_b2 = sb.tile([128, D], bf16, tag="wob", name="wo_b2")
            nc.sync.dma_start(out=wo_a, in_=woT[0:128, :])
            nc.sync.dma_start(out=wo_b2[0:64, :], in_=woT[128:DH, :])
            nc.sync.dma_start(out=wo_b2[64:128, :], in_=woT[128:DH, :])
            ones128 = sb.tile([1, 128], bf16, tag="o1", name="ones128")
            nc.vector.memset(ones128, 1.0)
            outTr = outT.rearrange("(o p) s -> p o s", p=128)

            # mask broadcast AP over the 3 heads (stride-0 middle dim)
            m_ap = mask_sb[:, :]
            mask_b = bass.AP(tensor=m_ap.tensor, offset=m_ap.offset,
                             ap=[list(m_ap.ap[0]), [0, HPC], list(m_ap.ap[1])])

            # ---- qkv storage ----
            # QK8: fp8 DoubleRow layout. [part = 32*h + p, j(0=Q,1=K),
            # slot sl, token]; hd = 32*sl + p. Heads at partition bands
            # 0:32 / 32:64 / 64:96.
            QK8 = sb.tile([128, 2, 2, S], fp8, tag="qk8", name="QK8")
            V_sb = sb.tile([128, NKC, HPC, HD + 1], bf16, tag="vsb", name="V_sb")
            nc.vector.memset(V_sb[:, :, :, HD:HD + 1], 1.0)

            sps = ctx.enter_context(tc.tile_pool(name="sps", bufs=2, space="PSUM"))
            cps = ctx.enter_context(tc.tile_pool(name="cps", bufs=1, space="PSUM"))
            att = ctx.enter_context(tc.tile_pool(name="att", bufs=18))
            nrm = ctx.enter_context(tc.tile_pool(name="nrm", bufs=2))
            stg = ctx.enter_context(tc.tile_pool(name="stg", bufs=2))
            q8s = ctx.enter_context(tc.tile_pool(name="q8s", bufs=2))

            def grab_cab():
                # ctx banks, either PV accumulators or projection scratch.
                # cA spans a full 2KB bank (512 f32) so 512-wide projection
                # outputs need a single accumulation group + single drain.
                cA = cps.tile([128, 512], f32, tag="cA", name="cA")
                cB = cps.tile([128, 512], f32, tag="cB", name="cB")
                return cA, cB

            def strided(base_ap, stride, count, width):
                # [128, count, width] AP with a custom free-dim stride
                return bass.AP(tensor=base_ap.tensor, offset=base_ap.offset,
                               ap=[list(base_ap.ap[0]), [stride, count],
                                   [1, width]])

            def bcast(base_ap, count, width):
                # [128, count, width] AP broadcasting each of `count` scalars
                return bass.AP(tensor=base_ap.tensor, offset=base_ap.offset,
                               ap=[list(base_ap.ap[0]), [1, count],
                                   [0, width]])

            qk_stg = {}

            def emit_repart1(n):
                # unpaired repartition of one qsb (startup latency path)
                st = qk_stg[n // 2]
                par = n % 2
                cols = slice(n * QSB, (n + 1) * QSB)
                for j in range(2):
                    for sl in range(2):
                        nc.sync.dma_start(
                            out=QK8[0:64, j, sl, cols],
                            in_=st[64 * sl:64 * sl + 64, j, par, :])
                for j in range(2):
                    for sl in range(2):
                        nc.sync.dma_start(
                            out=QK8[64:96, j, sl, cols],
                            in_=st[64 * j + 32 * sl:64 * j + 32 * sl + 32,
                                   2, par, :])

            def emit_qk_mm(n, i, k, bank):
                # pass i: 0 -> Q h0|h1, 1 -> K h0|h1, 2 -> Q h2 | K h2.
                # Pass output partition order:
                # i<2: 64*sl + 32*h + p ; i==2: 64*j + 32*sl + p
                if i == 0 and k == 0 and n % 2 == 0:
                    qk_stg[n // 2] = q8s.tile([128, HPC, 2, QSB], fp8,
                                              tag="q8", name="q8stg")
                cc = slice(n * QSB, (n + 1) * QSB)
                nc.tensor.matmul(bank[:, 0:QSB], wqk_sb[:, k, i, :],
                                 xt[k][:, cc], start=(k == 0),
                                 stop=(k == KD - 1))

            def emit_qk_drain(n, i, bank):
                # drains write fp8 staging (paired over two qsbs) for the
                # DoubleRow repartition
                st = qk_stg[n // 2]
                par = n % 2
                if QK_DRAIN_ENG[i] == "A":
                    nc.scalar.copy(st[:, i, par, :], bank[:, 0:QSB])
                else:
                    nc.vector.tensor_scalar_add(st[:, i, par, :],
                                                bank[:, 0:QSB],
                                                qkb_sb[:, i:i + 1])

            def emit_qk_pass(n, i, bank):
                for k in range(KD):
                    emit_qk_mm(n, i, k, bank)
                emit_qk_drain(n, i, bank)

            def emit_repart(n):
                # staging pair (n-1, n) -> QK8 DoubleRow bands, 8 DMAs
                st = qk_stg.pop(n // 2)
                cols = slice((n - 1) * QSB, (n + 1) * QSB)
                for j in range(2):          # pass j: Q / K of heads 0,1
                    for sl in range(2):
                        nc.sync.dma_start(
                            out=QK8[0:64, j, sl, cols],
                            in_=st[64 * sl:64 * sl + 64, j, :, :])
                for j in range(2):          # pass 2: h2 Q top | K bottom
                    for sl in range(2):
                        nc.sync.dma_start(
                            out=QK8[64:96, j, sl, cols],
                            in_=st[64 * j + 32 * sl:64 * j + 32 * sl + 32,
                                   2, :, :])

            def emit_v_one(n, j, bank):
                # V for one 128-token chunk j of token chunk n
                t = 4 * n + j
                po = bank[:, (j % 2) * 192:(j % 2) * 192 + DH]
                tcols = slice(t * 128, (t + 1) * 128)
                for k in range(KD):
                    nc.tensor.matmul(po, xt[k][:, tcols], wv_sb[:, k, :],
                                     start=(k == 0 and j % 2 == 0),
                                     stop=False, skip_group_check=True)
                nc.tensor.matmul(po, ones128, bv_sb, start=False,
                                 stop=(j % 2 == 1),
                                 skip_group_check=True)
                if V_COPY_ENG == "A":
                    nc.scalar.copy(V_sb[:, t, :, 0:HD],
                                   po.rearrange("p (h d) -> p h d", h=HPC))
                else:
                    nc.vector.tensor_copy(
                        V_sb[:, t, :, 0:HD],
                        po.rearrange("p (h d) -> p h d", h=HPC))

            def emit_v_pair(n, jp, bank):
                emit_v_one(n, 2 * jp, bank)
                emit_v_one(n, 2 * jp + 1, bank)

            def emit_outproj_oc(m, cnaq, cnb2, i, osb, bank):
                # output projection for qsb m, dout chunk i; all four
                # q-blocks land in one bank, one drain instr
                cA = bank
                oc = slice(i * 128, (i + 1) * 128)
                for qs in range(4):
                    po = cA[:, qs * 128:(qs + 1) * 128]
                    b0 = 64 * (qs % 2)
                    nc.tensor.matmul(po, wo_a[:, oc], cnaq[:, qs, :],
                                     start=(qs == 0), stop=False,
                                     skip_group_check=True)
                    nc.tensor.matmul(po, wo_b2[b0:b0 + 64, oc],
                                     cnb2[b0:b0 + 64, qs // 2, :],
                                     start=False, stop=(qs == 3),
                                     skip_group_check=True)
                if i in OUTPROJ_ACT:
                    nc.scalar.copy(osb[:, i, :], cA[:, 0:QSB])
                else:
                    nc.vector.tensor_copy(osb[:, i, :], cA[:, 0:QSB])
                last = m == NQSB - 1
                if last and i == 2:
                    nc.sync.dma_start(
                        out=outTr[:, 0:3, m * QSB:(m + 1) * QSB],
                        in_=osb[:, 0:3, :])
                if i == 5:
                    if last:
                        nc.sync.dma_start(
                            out=outTr[:, 3:6, m * QSB:(m + 1) * QSB],
                            in_=osb[:, 3:6, :])
                    else:
                        nc.sync.dma_start(
                            out=outTr[:, :, m * QSB:(m + 1) * QSB], in_=osb)

            def emit_transposes(pA, pB, cnaq, cnb2, bank):
                cAt = bank
                srcs = [pA[:, qs, :, :] for qs in range(4)]
                srcs += [pB[:, 2 * p:2 * p + 2, :] for p in range(2)]
                dsts = [cnaq[:, qs, :] for qs in range(4)]
                dsts += [cnb2[:, p, :] for p in range(2)]
                for i, (src, dst) in enumerate(zip(srcs, dsts)):
                    out = cAt[:, i * 64:(i + 1) * 64].bitcast(bf16)
                    nc.tensor.matmul(out, src, id_sb[:, :],
                                     is_transpose=True, start=(i == 0),
                                     stop=(i == 5), skip_group_check=True)
                    nc.vector.tensor_copy(dst, out)

            keep = {}
            keep_cn = {}
            outproj_sched = {4: [0], 5: [1, 2], 6: [3, 4], 7: [5, 6]}
            # QKV projections front-loaded into the engine-idle early
            # windows; windows 3+ are purely attention (+outproj) paced
            proj_sched = {0: [2], 1: [3], 2: [4], 3: [5], 4: [6], 5: [7]}
            pre_map = {0: 4, 1: 8, 2: 12, 3: 16, 4: 12, 5: 14, 6: 16, 7: 16}

            # projections for token chunks 0-1 run before the first qsb
            # (overlapping the input DMA trickle); scratch ping-pongs
            # between the two ctx banks so PE never waits on drains
            pro_pair = grab_cab()
            ptog = [0]

            def pbank():
                b = pro_pair[ptog[0] % 2]
                ptog[0] += 1
                return b

            emit_qk_pass(0, 0, pbank())
            emit_qk_pass(0, 2, pbank())
            emit_qk_pass(0, 1, pbank())
            # x tails staged so early columns land first: qsb1, qsb2-3,
            # then the bulk — keeps the prologue/window-0 projections fed
            for k in range(KD):
                nc.sync.dma_start(out=xt[k][:, QSB:2 * QSB],
                                  in_=xT[k * 128:(k + 1) * 128, QSB:2 * QSB])
            for k in range(KD):
                nc.sync.dma_start(out=xt[k][:, 2 * QSB:S],
                                  in_=xT[k * 128:(k + 1) * 128, 2 * QSB:S])
            emit_repart1(0)
            emit_qk_pass(1, 0, pbank())
            emit_qk_pass(1, 2, pbank())
            emit_qk_pass(1, 1, pbank())
            emit_repart1(1)
            qk_stg.pop(0)

            for n in range(NQSB):
                nkc = 4 * n + 4
                q0 = n * QSB
                pre = min(nkc, pre_map[n])

                ctxnA = nrm.tile([128, 4, 2, HD], bf16, tag="ctxnA",
                                 name="ctxnA")
                ctxnB = nrm.tile([128, 4, HD], bf16, tag="ctxnB", name="ctxnB")
                rcp = nrm.tile([128, 4 * HPC], f32, tag="rcp", name="rcp")

                eac = EXP_ACT_COLS

                def emit_scores(kc):
                    off = max(0, kc - 4 * n) * 128
                    sp = sps.tile([128, HPC * QSB], f32, tag="sp", name="sp")
                    kk = slice(kc * 128, (kc + 1) * 128)
                    pieces = [(a, b) for (a, b) in ((off, 256),
                                                    (max(off, 256), QSB))
                              if a < b]
                    for h in range(HPC):
                        b0 = 32 * h
                        for pi, (a, bnd) in enumerate(pieces):
                            qq = slice(q0 + a, q0 + bnd)
                            nc.tensor.matmul(
                                sp[:, h * QSB + a:h * QSB + bnd],
                                QK8[b0:b0 + 32, 1, :, kk],
                                QK8[b0:b0 + 32, 0, :, qq],
                                start=(pi == 0), stop=(bnd == QSB),
                                skip_group_check=(pi > 0),
                                perf_mode=DR)
                    es = att.tile([128, HPC * QSB], bf16, tag="es", name="es")
                    if off == 0:
                        # full chunk: flat split Act | DVE
                        nc.scalar.activation(es[:, 0:eac],
                                             sp[:, 0:eac], EXP,
                                             scale=0.125)
                        nc.vector.tensor_scalar(
                            es[:, eac:].bitcast(i16),
                            sp[:, eac:], SCH_A, SCH_B, MUL, ADD)
                    else:
                        # diag chunk: Act takes h0|h1 (strided), DVE h2
                        w = QSB - off
                        nc.scalar.activation(
                            strided(es[:, off:], QSB, 2, w),
                            strided(sp[:, off:], QSB, 2, w), EXP, scale=0.125)
                        nc.vector.tensor_scalar(
                            es[:, 2 * QSB + off:].bitcast(i16),
                            sp[:, 2 * QSB + off:], SCH_A, SCH_B, MUL, ADD)
                    if kc >= 4 * n:
                        msl = strided(es[:, off:], QSB, HPC, 128)
                        nc.vector.tensor_tensor(msl, msl, mask_b, MUL)
                    return es

                def emit_pv(kc, es, cA, cB, started):
                    off = max(0, kc - 4 * n) * 128
                    for h in range(HPC):
                        for qs in range(off // 128, 4):
                            g = h * 4 + qs
                            bank = int(g >= 7)
                            sl = (cA[:, g * 65:g * 65 + 65] if g < 7
                                  else cB[:, (g - 7) * 65:(g - 7) * 65 + 65])
                            st = kc == 0 and bank not in started
                            if st:
                                started.add(bank)
                            nc.tensor.matmul(
                                sl,
                                es[:, h * QSB + qs * 128:
                                   h * QSB + (qs + 1) * 128],
                                V_sb[:, kc, h, :],
                                start=st, stop=(kc == 4 * n + qs),
                                skip_group_check=True)

                def emit_normalize(cA, cB):
                    # batched: denominators live at col 64 of each 65-wide
                    # qsub slot; rcp[g] = 1/den, then broadcast-multiply
                    da = cA[:, 64:65]
                    db = cB[:, 64:65]
                    nc.vector.reciprocal(
                        rcp[:, 0:7],
                        bass.AP(tensor=da.tensor, offset=da.offset,
                                ap=[list(da.ap[0]), [65, 7]]))
                    nc.vector.reciprocal(
                        rcp[:, 7:12],
                        bass.AP(tensor=db.tensor, offset=db.offset,
                                ap=[list(db.ap[0]), [65, 5]]))
                    # h0: cA slots 0-3 -> ctxnA[:, :, 0, :]
                    nc.vector.tensor_tensor(
                        strided(ctxnA[:, 0, 0, :], 2 * HD, 4, HD),
                        strided(cA[:, 0:], 65, 4, HD),
                        bcast(rcp[:, 0:], 4, HD), MUL)
                    # h1 j0-2: cA slots 4-6 -> ctxnA[:, 0:3, 1, :]
                    nc.vector.tensor_tensor(
                        strided(ctxnA[:, 0, 1, :], 2 * HD, 3, HD),
                        strided(cA[:, 4 * 65:], 65, 3, HD),
                        bcast(rcp[:, 4:], 3, HD), MUL)
                    # h1 j3: cB slot 0 -> ctxnA[:, 3, 1, :]
                    nc.vector.tensor_tensor(
                        ctxnA[:, 3, 1, :], cB[:, 0:HD],
                        bcast(rcp[:, 7:], 1, HD), MUL)
                    # h2: cB slots 1-4 -> ctxnB
                    nc.vector.tensor_tensor(
                        strided(ctxnB[:, 0, :], HD, 4, HD),
                        strided(cB[:, 65:], 65, 4, HD),
                        bcast(rcp[:, 8:], 4, HD), MUL)

                # scratch-window work: transposes + output projection for
                # qsb n-1, projection for chunk n+1; items ping-pong
                # between the two ctx banks
                cA, cB = grab_cab()
                wpair = (cA, cB)
                wtog = [0]

                def wbank():
                    b = wpair[wtog[0] % 2]
                    wtog[0] += 1
                    return b

                work = []
                if n > 0:
                    cnaq = nrm.tile([128, 4, 128], bf16, tag="cna", bufs=8,
                                    name="cnaq")
                    cnb2 = nrm.tile([128, 2, 128], bf16, tag="cnb", bufs=8,
                                    name="cnb2")
                    keep_cn[n - 1] = (cnaq, cnb2)
                    pA, pB = keep[n - 1]
                    work.append(lambda a=cnaq, b=cnb2, pa=pA, pb=pB:
                                emit_transposes(pa, pb, a, b, wbank()))
                if n == 0:
                    for m0 in (0, 1):
                        for jp in range(2):
                            bk = wbank()
                            for j in (2 * jp, 2 * jp + 1):
                                work.append(lambda m0=m0, j=j, bk=bk:
                                            emit_v_one(m0, j, bk))
                for m in proj_sched.get(n, ()):
                    for i in (0, 2, 1):
                        bk = wbank()
                        for k in range(KD):
                            work.append(lambda m=m, i=i, k=k, bk=bk:
                                        emit_qk_mm(m, i, k, bk))
                        work.append(lambda m=m, i=i, bk=bk:
                                    emit_qk_drain(m, i, bk))
                    if m % 2 == 1:
                        work.append(lambda m=m: emit_repart(m))
                    for jp in range(2):
                        bk = wbank()
                        for j in (2 * jp, 2 * jp + 1):
                            work.append(lambda m=m, j=j, bk=bk:
                                        emit_v_one(m, j, bk))
                for m in outproj_sched.get(n, ()):
                    osb = stg.tile([128, 6, QSB], f16, tag="osb", name="osb")
                    a, b = keep_cn[m]
                    for i in range(6):
                        work.append(
                            lambda a=a, b=b, i=i, m=m, o=osb:
                            emit_outproj_oc(m, a, b, i, o, wbank()))

                # ---- pre-PV window: scores stream + scratch work ----
                es_q = []
                for kc in range(pre):
                    es_q.append(emit_scores(kc))
                    left = pre - kc - 1
                    npop = (len(work) if left == 0
                            else -(-len(work) // (left + 1)))
                    for _ in range(min(npop, len(work))):
                        work.pop(0)()
                while work:
                    work.pop(0)()
                # ---- P@V catch-up, then steady skewed loop ----
                started = set()
                backlog = list(range(pre))
                if pre == nkc:
                    for kc in backlog:
                        emit_pv(kc, es_q[kc], cA, cB, started)
                    backlog = []
                for kc in range(pre, nkc):
                    es_q.append(emit_scores(kc))
                    if backlog:
                        j = backlog.pop(0)
                        emit_pv(j, es_q[j], cA, cB, started)
                    if kc > pre:
                        emit_pv(kc - 1, es_q[kc - 1], cA, cB, started)
                for j in backlog:
                    emit_pv(j, es_q[j], cA, cB, started)
                if nkc > pre:
                    emit_pv(nkc - 1, es_q[nkc - 1], cA, cB, started)
                emit_normalize(cA, cB)
                keep[n] = (ctxnA, ctxnB)
            # tail: transposes + output projection of the last qsb
            cnaq = nrm.tile([128, 4, 128], bf16, tag="cna", bufs=8,
                            name="cnaq")
            cnb2 = nrm.tile([128, 2, 128], bf16, tag="cnb", bufs=8,
                            name="cnb2")
            tl_pair = grab_cab()
            emit_transposes(*keep[NQSB - 1], cnaq, cnb2, tl_pair[0])
            osb = stg.tile([128, 6, QSB], f16, tag="osb", name="osb")
            for i in range(6):
                emit_outproj_oc(NQSB - 1, cnaq, cnb2, i, osb,
                                tl_pair[(i + 1) % 2])
    nc.compile()
    return nc


def _in_maps(x, Wq, bq, Wk, bk, Wv, bv, Wo, bo):
    tri = np.triu(np.ones((128, 128), np.float32)).astype(BF)
    WqT, WkT, WvT = Wq.T, Wk.T, Wv.T
    maps = []
    for c in range(N_CORES):
        b, hg = c // 4, c % 4
        sl = slice(DH * hg, DH * hg + DH)
        wqk = np.empty((D, HPC, 128), np.float32)
        qkb = np.zeros((128, HPC), np.float32)
        h0, h1, h2 = (DH * hg + HD * h for h in range(HPC))
        # passes 0/1 partition order: 64*sl + 32*h + p (h in {0,1});
        # pass 2: 64*j + 32*sl + p (j: 0=Q, 1=K of h2)
        for slot in range(2):
            o = 32 * slot
            wqk[:, 0, 64 * slot + 0:64 * slot + 32] = WqT[:, h0 + o:h0 + o + 32]
            wqk[:, 0, 64 * slot + 32:64 * slot + 64] = WqT[:, h1 + o:h1 + o + 32]
            wqk[:, 1, 64 * slot + 0:64 * slot + 32] = WkT[:, h0 + o:h0 + o + 32]
            wqk[:, 1, 64 * slot + 32:64 * slot + 64] = WkT[:, h1 + o:h1 + o + 32]
            wqk[:, 2, 32 * slot:32 * slot + 32] = WqT[:, h2 + o:h2 + o + 32]
            wqk[:, 2, 64 + 32 * slot:64 + 32 * slot + 32] = WkT[:, h2 + o:h2 + o + 32]
            qkb[64 * slot + 0:64 * slot + 32, 0] = bq[h0 + o:h0 + o + 32]
            qkb[64 * slot + 32:64 * slot + 64, 0] = bq[h1 + o:h1 + o + 32]
            qkb[64 * slot + 0:64 * slot + 32, 1] = bk[h0 + o:h0 + o + 32]
            qkb[64 * slot + 32:64 * slot + 64, 1] = bk[h1 + o:h1 + o + 32]
            qkb[32 * slot:32 * slot + 32, 2] = bq[h2 + o:h2 + o + 32]
            qkb[64 + 32 * slot:64 + 32 * slot + 32, 2] = bk[h2 + o:h2 + o + 32]
        maps.append({
            "xT": np.ascontiguousarray(x[b].T).astype(BF),
            "wqk": wqk.astype(BF),
            "wv": np.ascontiguousarray(WvT[:, sl]).astype(BF),
            "woT": np.ascontiguousarray(Wo[:, sl].T).astype(BF),
            "qkb": qkb,
            "bvv": bv[sl].reshape(1, DH).astype(BF),
            "tmk": tri,
            "idm": np.eye(128, dtype=np.float32).astype(BF),
        })
    return maps


def kernel(x, Wq, bq, Wk, bk, Wv, bv, Wo, bo):
    if "nc" not in _CACHE:
        _CACHE["nc"] = build()
    nc = _CACHE["nc"]
    maps = _in_maps(x, Wq, bq, Wk, bk, Wv, bv, Wo, bo)
    res = run_bass_kernel_spmd(nc, maps, list(range(N_CORES))).results
    out = np.zeros((B, S, D), np.float32)
    for c in range(N_CORES):
        out[c // 4] += res[c]["outT"].astype(np.float32).T
    out += bo.astype(np.float32)
    return out


# revision 33
# speedup vs baseline: 1.0118x; 1.0118x over previous
"""GPT causal attention (B=2,S=4096,D=768,H=12,HD=64) on 8 NeuronCores.

Sharding: core c handles batch b=c//4 and head-group hg=c%4 (3 heads each);
the host sums the 4 head-group partials per batch and adds bo.

Per-core kernel (fp32 PSUM; fp8 scores, bf16 elsewhere, f16 output):
  - Q/K kept in fp8e4m3 DoubleRow layout [32, 2, S] per head (hd=32*slot+p),
    built by pair-of-qsb partition-shift DMAs from fp8 projection drains;
    the scores matmul runs in DoubleRow perf mode (0.5 cycles/row) into a
    flat 3-head fused PSUM tile [128, 1536]
  - softmax exp SPLIT across engines: Act runs exact exp on the first
    EXP_ACT_COLS flat columns per key chunk, DVE runs a Schraudolph
    bit-trick exp (i16 = s*0.125*log2e*128 + C, bitcast bf16) on the rest;
    the causal mask multiply stays on DVE (a GpSimd hop in the
    diag-chunk -> P@V chain cost ~8us; GpSimd also cannot read PSUM)
  - P@V: es is the STATIONARY operand, V [128,65] the moving one; col 64
    of V is ones so ctx PSUM col 64 accumulates the softmax denominator
  - normalization batched per window: 2 strided reciprocals + 4 broadcast
    tensor_tensor multiplies drain all 12 ctx qsub slots at once
  - ctx transposed to head-dim-major via PE pair-transposes; output
    projection accumulates all 4 q-blocks in one full PSUM bank (single
    drain instr), staged f16 and stored f16
  - PSUM: scores double-buffer 6 banks + 2 ctx banks. The QKV / output
    projections run in a pre-PV window (P@V deferred, es tiles buffered)
    REUSING the ctx banks as scratch; scratch items are emitted at
    per-matmul granularity and PING-PONG between the two banks so PE
    never head-blocks on a drain or a stalled item (worth ~30us).
"""
import contextlib
import sys

sys.path.insert(0, "/opt/trn_rl_repo")

import numpy as np
import ml_dtypes

import concourse.bass as bass
import concourse.tile as tile
from concourse import bacc, mybir
from concourse.bass_utils import run_bass_kernel_spmd

B, S, D, H, HD = 2, 4096, 768, 12, 64
N_CORES = 8
HPC = 3           # heads per core
DH = HPC * HD     # 192 per-core head dims
KD = D // 128     # 6 contraction chunks
QSB = 512         # query superblock width
NQSB = S // QSB   # 8
NKC = S // 128    # 32 key chunks

f32 = mybir.dt.float32
f16 = mybir.dt.float16
bf16 = mybir.dt.bfloat16
i16 = mybir.dt.int16
fp8 = mybir.dt.float8e4
BF = ml_dtypes.bfloat16
EXP = mybir.ActivationFunctionType.Exp
MUL = mybir.AluOpType.mult
ADD = mybir.AluOpType.add
DR = mybir.MatmulPerfMode.DoubleRow

# Schraudolph exp constants (bf16 bit domain, folds the 1/sqrt(HD) scale)
L2E = 1.4426950408889634
SCH_A = 0.125 * L2E * 128.0
SCH_C = 4.43
SCH_B = 16256.0 - SCH_C

# flat columns (of 3*512 per key chunk) handled by Act's exact exp; the
# remainder goes to DVE's Schraudolph
EXP_ACT_COLS = 1024
# outproj drain dout-chunks handled by Act (rest DVE)
OUTPROJ_ACT = (0, 3)
# engine for the QKV projection drains: "A" or "D" per pass index
QK_DRAIN_ENG = ("D", "D", "D")
V_COPY_ENG = "D"

_CACHE = {}


def build():
    nc = bacc.Bacc("TRN2", target_bir_lowering=False, debug=False,
                   num_devices=N_CORES)
    xT = nc.dram_tensor("xT", [D, S], bf16, kind="ExternalInput").ap()
    wqk = nc.dram_tensor("wqk", [D, HPC, 128], bf16, kind="ExternalInput").ap()
    wv = nc.dram_tensor("wv", [D, DH], bf16, kind="ExternalInput").ap()
    woT = nc.dram_tensor("woT", [DH, D], bf16, kind="ExternalInput").ap()
    qkb = nc.dram_tensor("qkb", [128, HPC], f32, kind="ExternalInput").ap()
    bvv = nc.dram_tensor("bvv", [1, DH], bf16, kind="ExternalInput").ap()
    tmk = nc.dram_tensor("tmk", [128, 128], bf16, kind="ExternalInput").ap()
    idm = nc.dram_tensor("idm", [128, 128], bf16, kind="ExternalInput").ap()
    outT = nc.dram_tensor("outT", [D, S], f16, kind="ExternalOutput").ap()

    with tile.TileContext(nc) as tc:
        with contextlib.ExitStack() as ctx:
            sb = ctx.enter_context(tc.tile_pool(name="sb", bufs=1))
            # ---- resident inputs ----
            wqk_sb = sb.tile([128, KD, HPC, 128], bf16, tag="wqk", name="wqk_sb")
            nc.sync.dma_start(
                out=wqk_sb, in_=wqk.rearrange("(ko p) h m -> p ko h m", p=128))
            xt = [sb.tile([128, S], bf16, tag=f"xt{k}", name=f"xt{k}")
                  for k in range(KD)]
            for k in range(KD):
                nc.sync.dma_start(out=xt[k][:, 0:QSB],
                                  in_=xT[k * 128:(k + 1) * 128, 0:QSB])
            qkb_sb = sb.tile([128, HPC], f32, tag="qkb", name="qkb_sb")
            nc.sync.dma_start(out=qkb_sb, in_=qkb)
            wv_sb = sb.tile([128, KD, DH], bf16, tag="wv", name="wv_sb")
            nc.sync.dma_start(
                out=wv_sb, in_=wv.rearrange("(ko p) m -> p ko m", p=128))
            bv_sb = sb.tile([1, DH], bf16, tag="bv", name="bv_sb")
            nc.sync.dma_start(out=bv_sb, in_=bvv)
            mask_sb = sb.tile([128, 128], bf16, tag="mk", name="mask_sb")
            nc.sync.dma_start(out=mask_sb, in_=tmk)
            id_sb = sb.tile([128, 128], bf16, tag="id", name="id_sb")
            nc.sync.dma_start(out=id_sb, in_=idm)
            wo_a = sb.tile([128, D], bf16, tag="woa", name="wo_a")
            # head-2 rows of woT duplicated into BOTH partition halves, so
            # either half of a pair-transposed cnb tile can contract with it
            wo_b2 = sb.tile([128, D], bf16, tag="wob", name="wo_b2")
            nc.sync.dma_start(out=wo_a, in_=woT[0:128, :])
            nc.sync.dma_start(out=wo_b2[0:64, :], in_=woT[128:DH, :])
            nc.sync.dma_start(out=wo_b2[64:128, :], in_=woT[128:DH, :])
            ones128 = sb.tile([1, 128], bf16, tag="o1", name="ones128")
            nc.vector.memset(ones128, 1.0)
            outTr = outT.rearrange("(o p) s -> p o s", p=128)

            # mask broadcast AP over the 3 heads (stride-0 middle dim)
            m_ap = mask_sb[:, :]
            mask_b = bass.AP(tensor=m_ap.tensor, offset=m_ap.offset,
                             ap=[list(m_ap.ap[0]), [0, HPC], list(m_ap.ap[1])])

            # ---- qkv storage ----
            # QK8: fp8 DoubleRow layout. [part = 32*h + p, j(0=Q,1=K),
            # slot sl, token]; hd = 32*sl + p. Heads at partition bands
            # 0:32 / 32:64 / 64:96.
            QK8 = sb.tile([128, 2, 2, S], fp8, tag="qk8", name="QK8")
            V_sb = sb.tile([128, NKC, HPC, HD + 1], bf16, tag="vsb", name="V_sb")
            nc.vector.memset(V_sb[:, :, :, HD:HD + 1], 1.0)

            sps = ctx.enter_context(tc.tile_pool(name="sps", bufs=2, space="PSUM"))
            cps = ctx.enter_context(tc.tile_pool(name="cps", bufs=1, space="PSUM"))
            att = ctx.enter_context(tc.tile_pool(name="att", bufs=18))
            nrm = ctx.enter_context(tc.tile_pool(name="nrm", bufs=2))
            stg = ctx.enter_context(tc.tile_pool(name="stg", bufs=2))
            q8s = ctx.enter_context(tc.tile_pool(name="q8s", bufs=2))

            def grab_cab():
                # ctx banks, either PV accumulators or projection scratch.
                # cA spans a full 2KB bank (512 f32) so 512-wide projection
                # outputs need a single accumulation group + single drain.
                cA = cps.tile([128, 512], f32, tag="cA", name="cA")
                cB = cps.tile([128, 512], f32, tag="cB", name="cB")
                return cA, cB

            def strided(base_ap, stride, count, width):
                # [128, count, width] AP with a custom free-dim stride
                return bass.AP(tensor=base_ap.tensor, offset=base_ap.offset,
                               ap=[list(base_ap.ap[0]), [stride, count],
                                   [1, width]])

            def bcast(base_ap, count, width):
                # [128, count, width] AP broadcasting each of `count` scalars
                return bass.AP(tensor=base_ap.tensor, offset=base_ap.offset,
                               ap=[list(base_ap.ap[0]), [1, count],
                                   [0, width]])

            qk_stg = {}

            def emit_repart1(n):
                # unpaired repartition of one qsb (startup latency path)
                st = qk_stg[n // 2]
                par = n % 2
                cols = slice(n * QSB, (n + 1) * QSB)
                for j in range(2):
                    for sl in range(2):
                        nc.sync.dma_start(
                            out=QK8[0:64, j, sl, cols],
                            in_=st[64 * sl:64 * sl + 64, j, par, :])
                for j in range(2):
                    for sl in range(2):
                        nc.sync.dma_start(
                            out=QK8[64:96, j, sl, cols],
                            in_=st[64 * j + 32 * sl:64 * j + 32 * sl + 32,
                                   2, par, :])

            def emit_qk_mm(n, i, k, bank):
                # pass i: 0 -> Q h0|h1, 1 -> K h0|h1, 2 -> Q h2 | K h2.
                # Pass output partition order:
                # i<2: 64*sl + 32*h + p ; i==2: 64*j + 32*sl + p
                if i == 0 and k == 0 and n % 2 == 0:
                    qk_stg[n // 2] = q8s.tile([128, HPC, 2, QSB], fp8,
                                              tag="q8", name="q8stg")
                cc = slice(n * QSB, (n + 1) * QSB)
                nc.tensor.matmul(bank[:, 0:QSB], wqk_sb[:, k, i, :],
                                 xt[k][:, cc], start=(k == 0),
                                 stop=(k == KD - 1))

            def emit_qk_drain(n, i, bank):
                # drains write fp8 staging (paired over two qsbs) for the
                # DoubleRow repartition
                st = qk_stg[n // 2]
                par = n % 2
                if QK_DRAIN_ENG[i] == "A":
                    nc.scalar.copy(st[:, i, par, :], bank[:, 0:QSB])
                else:
                    nc.vector.tensor_scalar_add(st[:, i, par, :],
                                                bank[:, 0:QSB],
                                                qkb_sb[:, i:i + 1])

            def emit_qk_pass(n, i, bank):
                for k in range(KD):
                    emit_qk_mm(n, i, k, bank)
                emit_qk_drain(n, i, bank)

            def emit_repart(n):
                # staging pair (n-1, n) -> QK8 DoubleRow bands, 8 DMAs
                st = qk_stg.pop(n // 2)
                cols = slice((n - 1) * QSB, (n + 1) * QSB)
                for j in range(2):          # pass j: Q / K of heads 0,1
                    for sl in range(2):
                        nc.sync.dma_start(
                            out=QK8[0:64, j, sl, cols],
                            in_=st[64 * sl:64 * sl + 64, j, :, :])
                for j in range(2):          # pass 2: h2 Q top | K bottom
                    for sl in range(2):
                        nc.sync.dma_start(
                            out=QK8[64:96, j, sl, cols],
                            in_=st[64 * j + 32 * sl:64 * j + 32 * sl + 32,
                                   2, :, :])

            def emit_v_one(n, j, bank):
                # V for one 128-token chunk j of token chunk n
                t = 4 * n + j
                po = bank[:, (j % 2) * 192:(j % 2) * 192 + DH]
                tcols = slice(t * 128, (t + 1) * 128)
                for k in range(KD):
                    nc.tensor.matmul(po, xt[k][:, tcols], wv_sb[:, k, :],
                                     start=(k == 0 and j % 2 == 0),
                                     stop=False, skip_group_check=True)
                nc.tensor.matmul(po, ones128, bv_sb, start=False,
                                 stop=(j % 2 == 1),
                                 skip_group_check=True)
                if V_COPY_ENG == "A":
                    nc.scalar.copy(V_sb[:, t, :, 0:HD],
                                   po.rearrange("p (h d) -> p h d", h=HPC))
                else:
                    nc.vector.tensor_copy(
                        V_sb[:, t, :, 0:HD],
                        po.rearrange("p (h d) -> p h d", h=HPC))

            def emit_v_pair(n, jp, bank):
                emit_v_one(n, 2 * jp, bank)
                emit_v_one(n, 2 * jp + 1, bank)

            def emit_outproj_oc(m, cnaq, cnb2, i, osb, bank):
                # output projection for qsb m, dout chunk i; all four
                # q-blocks land in one bank, one drain instr
                cA = bank
                oc = slice(i * 128, (i + 1) * 128)
                for qs in range(4):
                    po = cA[:, qs * 128:(qs + 1) * 128]
                    b0 = 64 * (qs % 2)
                    nc.tensor.matmul(po, wo_a[:, oc], cnaq[:, qs, :],
                                     start=(qs == 0), stop=False,
                                     skip_group_check=True)
                    nc.tensor.matmul(po, wo_b2[b0:b0 + 64, oc],
                                     cnb2[b0:b0 + 64, qs // 2, :],
                                     start=False, stop=(qs == 3),
                                     skip_group_check=True)
                if i in OUTPROJ_ACT:
                    nc.scalar.copy(osb[:, i, :], cA[:, 0:QSB])
                else:
                    nc.vector.tensor_copy(osb[:, i, :], cA[:, 0:QSB])
                last = m == NQSB - 1
                if last and i == 2:
                    nc.sync.dma_start(
                        out=outTr[:, 0:3, m * QSB:(m + 1) * QSB],
                        in_=osb[:, 0:3, :])
                if i == 5:
                    if last:
                        nc.sync.dma_start(
                            out=outTr[:, 3:6, m * QSB:(m + 1) * QSB],
                            in_=osb[:, 3:6, :])
                    else:
                        nc.sync.dma_start(
                            out=outTr[:, :, m * QSB:(m + 1) * QSB], in_=osb)

            def emit_transposes(pA, pB, cnaq, cnb2, bank):
                cAt = bank
                srcs = [pA[:, qs, :, :] for qs in range(4)]
                srcs += [pB[:, 2 * p:2 * p + 2, :] for p in range(2)]
                dsts = [cnaq[:, qs, :] for qs in range(4)]
                dsts += [cnb2[:, p, :] for p in range(2)]
                for i, (src, dst) in enumerate(zip(srcs, dsts)):
                    out = cAt[:, i * 64:(i + 1) * 64].bitcast(bf16)
                    nc.tensor.matmul(out, src, id_sb[:, :],
                                     is_transpose=True, start=(i == 0),
                                     stop=(i == 5), skip_group_check=True)
                    nc.vector.tensor_copy(dst, out)

            keep = {}
            keep_cn = {}
            outproj_sched = {4: [0], 5: [1, 2], 6: [3, 4], 7: [5, 6]}
            # QKV projections front-loaded into the engine-idle early
            # windows; windows 3+ are purely attention (+outproj) paced
            proj_sched = {0: [2], 1: [3], 2: [4], 3: [5], 4: [6], 5: [7]}
            pre_map = {0: 4, 1: 8, 2: 12, 3: 16, 4: 12, 5: 14, 6: 16, 7: 16}

            # projections for token chunks 0-1 run before the first qsb
            # (overlapping the input DMA trickle); scratch ping-pongs
            # between the two ctx banks so PE never waits on drains
            pro_pair = grab_cab()
            ptog = [0]

            def pbank():
                b = pro_pair[ptog[0] % 2]
                ptog[0] += 1
                return b

            emit_qk_pass(0, 0, pbank())
            emit_qk_pass(0, 2, pbank())
            emit_qk_pass(0, 1, pbank())
            # x tails staged so early columns land first: qsb1, qsb2-3,
            # then the bulk — keeps the prologue/window-0 projections fed
            for k in range(KD):
                nc.sync.dma_start(out=xt[k][:, QSB:2 * QSB],
                                  in_=xT[k * 128:(k + 1) * 128, QSB:2 * QSB])
            for k in range(KD):
                nc.sync.dma_start(out=xt[k][:, 2 * QSB:S],
                                  in_=xT[k * 128:(k + 1) * 128, 2 * QSB:S])
            emit_repart1(0)
            emit_qk_pass(1, 0, pbank())
            emit_qk_pass(1, 2, pbank())
            emit_qk_pass(1, 1, pbank())
            emit_repart1(1)
            qk_stg.pop(0)

            for n in range(NQSB):
                nkc = 4 * n + 4
                q0 = n * QSB
                pre = min(nkc, pre_map[n])

                ctxnA = nrm.tile([128, 4, 2, HD], bf16, tag="ctxnA",
                                 name="ctxnA")
                ctxnB = nrm.tile([128, 4, HD], bf16, tag="ctxnB", name="ctxnB")
                rcp = nrm.tile([128, 4 * HPC], f32, tag="rcp", name="rcp")

                eac = EXP_ACT_COLS

                def emit_scores(kc):
                    off = max(0, kc - 4 * n) * 128
                    sp = sps.tile([128, HPC * QSB], f32, tag="sp", name="sp")
                    kk = slice(kc * 128, (kc + 1) * 128)
                    pieces = [(a, b) for (a, b) in ((off, 256),
                                                    (max(off, 256), QSB))
                              if a < b]
                    for h in range(HPC):
                        b0 = 32 * h
                        for pi, (a, bnd) in enumerate(pieces):
                            qq = slice(q0 + a, q0 + bnd)
                            nc.tensor.matmul(
                                sp[:, h * QSB + a:h * QSB + bnd],
                                QK8[b0:b0 + 32, 1, :, kk],
                                QK8[b0:b0 + 32, 0, :, qq],
                                start=(pi == 0), stop=(bnd == QSB),
                                skip_group_check=(pi > 0),
                                perf_mode=DR)
                    es = att.tile([128, HPC * QSB], bf16, tag="es", name="es")
                    if off == 0:
                        # full chunk: flat split Act | DVE
                        nc.scalar.activation(es[:, 0:eac],
                                             sp[:, 0:eac], EXP,
                                             scale=0.125)
                        nc.vector.tensor_scalar(
                            es[:, eac:].bitcast(i16),
                            sp[:, eac:], SCH_A, SCH_B, MUL, ADD)
                    else:
                        # diag chunk: Act takes h0|h1 (strided), DVE h2
                        w = QSB - off
                        nc.scalar.activation(
                            strided(es[:, off:], QSB, 2, w),
                            strided(sp[:, off:], QSB, 2, w), EXP, scale=0.125)
                        nc.vector.tensor_scalar(
                            es[:, 2 * QSB + off:].bitcast(i16),
                            sp[:, 2 * QSB + off:], SCH_A, SCH_B, MUL, ADD)
                    if kc >= 4 * n:
                        msl = strided(es[:, off:], QSB, HPC, 128)
                        nc.vector.tensor_tensor(msl, msl, mask_b, MUL)
                    return es

                def emit_pv(kc, es, cA, cB, started):
                    off = max(0, kc - 4 * n) * 128
                    for h in range(HPC):
                        for qs in range(off // 128, 4):
                            g = h * 4 + qs
                            bank = int(g >= 7)
                            sl = (cA[:, g * 65:g * 65 + 65] if g < 7
                                  else cB[:, (g - 7) * 65:(g - 7) * 65 + 65])
                            st = kc == 0 and bank not in started
                            if st:
                                started.add(bank)
                            nc.tensor.matmul(
                                sl,
                                es[:, h * QSB + qs * 128:
                                   h * QSB + (qs + 1) * 128],
                                V_sb[:, kc, h, :],
                                start=st, stop=(kc == 4 * n + qs),
                                skip_group_check=True)

                def emit_normalize(cA, cB):
                    # batched: denominators live at col 64 of each 65-wide
                    # qsub slot; rcp[g] = 1/den, then broadcast-multiply
                    da = cA[:, 64:65]
                    db = cB[:, 64:65]
                    nc.vector.reciprocal(
                        rcp[:, 0:7],
                        bass.AP(tensor=da.tensor, offset=da.offset,
                                ap=[list(da.ap[0]), [65, 7]]))
                    nc.vector.reciprocal(
                        rcp[:, 7:12],
                        bass.AP(tensor=db.tensor, offset=db.offset,
                                ap=[list(db.ap[0]), [65, 5]]))
                    # h0: cA slots 0-3 -> ctxnA[:, :, 0, :]
                    nc.vector.tensor_tensor(
                        strided(ctxnA[:, 0, 0, :], 2 * HD, 4, HD),
                        strided(cA[:, 0:], 65, 4, HD),
                        bcast(rcp[:, 0:], 4, HD), MUL)
                    # h1 j0-2: cA slots 4-6 -> ctxnA[:, 0:3, 1, :]
                    nc.vector.tensor_tensor(
                        strided(ctxnA[:, 0, 1, :], 2 * HD, 3, HD),
                        strided(cA[:, 4 * 65:], 65, 3, HD),
                        bcast(rcp[:, 4:], 3, HD), MUL)
                    # h1 j3: cB slot 0 -> ctxnA[:, 3, 1, :]
                    nc.vector.tensor_tensor(
                        ctxnA[:, 3, 1, :], cB[:, 0:HD],
                        bcast(rcp[:, 7:], 1, HD), MUL)
                    # h2: cB slots 1-4 -> ctxnB
                    nc.vector.tensor_tensor(
                        strided(ctxnB[:, 0, :], HD, 4, HD),
                        strided(cB[:, 65:], 65, 4, HD),
                        bcast(rcp[:, 8:], 4, HD), MUL)

                # scratch-window work: transposes + output projection for
                # qsb n-1, projection for chunk n+1; items ping-pong
                # between the two ctx banks
                cA, cB = grab_cab()
                wpair = (cA, cB)
                wtog = [0]

                def wbank():
                    b = wpair[wtog[0] % 2]
                    wtog[0] += 1
                    return b

                work = []
                if n > 0:
                    cnaq = nrm.tile([128, 4, 128], bf16, tag="cna", bufs=8,
                                    name="cnaq")
                    cnb2 = nrm.tile([128, 2, 128], bf16, tag="cnb", bufs=8,
                                    name="cnb2")
                    keep_cn[n - 1] = (cnaq, cnb2)
                    pA, pB = keep[n - 1]
                    work.append(lambda a=cnaq, b=cnb2, pa=pA, pb=pB:
                                emit_transposes(pa, pb, a, b, wbank()))
                if n == 0:
                    for m0 in (0, 1):
                        for jp in range(2):
                            bk = wbank()
                            for j in (2 * jp, 2 * jp + 1):
                                work.append(lambda m0=m0, j=j, bk=bk:
                                            emit_v_one(m0, j, bk))
                for m in proj_sched.get(n, ()):
                    for i in (0, 2, 1):
                        bk = wbank()
                        for k in range(KD):
                            work.append(lambda m=m, i=i, k=k, bk=bk:
                                        emit_qk_mm(m, i, k, bk))
                        work.append(lambda m=m, i=i, bk=bk:
                                    emit_qk_drain(m, i, bk))
                    if m % 2 == 1:
                        work.append(lambda m=m: emit_repart(m))
                    for jp in range(2):
                        bk = wbank()
                        for j in (2 * jp, 2 * jp + 1):
                            work.append(lambda m=m, j=j, bk=bk:
                                        emit_v_one(m, j, bk))
                for m in outproj_sched.get(n, ()):
                    osb = stg.tile([128, 6, QSB], f16, tag="osb", name="osb")
                    a, b = keep_cn[m]
                    for i in range(6):
                        work.append(
                            lambda a=a, b=b, i=i, m=m, o=osb:
                            emit_outproj_oc(m, a, b, i, o, wbank()))

                # ---- pre-PV window: scores stream + scratch work ----
                es_q = []
                for kc in range(pre):
                    es_q.append(emit_scores(kc))
                    left = pre - kc - 1
                    npop = (len(work) if left == 0
                            else -(-len(work) // (left + 1)))
                    for _ in range(min(npop, len(work))):
                        work.pop(0)()
                while work:
                    work.pop(0)()
                # ---- P@V catch-up, then steady skewed loop ----
                started = set()
                backlog = list(range(pre))
                if pre == nkc:
                    for kc in backlog:
                        emit_pv(kc, es_q[kc], cA, cB, started)
                    backlog = []
                for kc in range(pre, nkc):
                    es_q.append(emit_scores(kc))
                    if backlog:
                        j = backlog.pop(0)
                        emit_pv(j, es_q[j], cA, cB, started)
                    if kc > pre:
                        emit_pv(kc - 1, es_q[kc - 1], cA, cB, started)
                for j in backlog:
                    emit_pv(j, es_q[j], cA, cB, started)
                if nkc > pre:
                    emit_pv(nkc - 1, es_q[nkc - 1], cA, cB, started)
                emit_normalize(cA, cB)
                keep[n] = (ctxnA, ctxnB)
            # tail: transposes + output projection of the last qsb
            cnaq = nrm.tile([128, 4, 128], bf16, tag="cna", bufs=8,
                            name="cnaq")
            cnb2 = nrm.tile([128, 2, 128], bf16, tag="cnb", bufs=8,
                            name="cnb2")
            tl_pair = grab_cab()
            emit_transposes(*keep[NQSB - 1], cnaq, cnb2, tl_pair[0])
            osb = stg.tile([128, 6, QSB], f16, tag="osb", name="osb")
            for i in range(6):
                emit_outproj_oc(NQSB - 1, cnaq, cnb2, i, osb,
                                tl_pair[(i + 1) % 2])
    nc.compile()
    return nc


def _in_maps(x, Wq, bq, Wk, bk, Wv, bv, Wo, bo):
    tri = np.triu(np.ones((128, 128), np.float32)).astype(BF)
    WqT, WkT, WvT = Wq.T, Wk.T, Wv.T
    maps = []
    for c in range(N_CORES):
        b, hg = c // 4, c % 4
        sl = slice(DH * hg, DH * hg + DH)
        wqk = np.empty((D, HPC, 128), np.float32)
        qkb = np.zeros((128, HPC), np.float32)
        h0, h1, h2 = (DH * hg + HD * h for h in range(HPC))
        # passes 0/1 partition order: 64*sl + 32*h + p (h in {0,1});
        # pass 2: 64*j + 32*sl + p (j: 0=Q, 1=K of h2)
        for slot in range(2):
            o = 32 * slot
            wqk[:, 0, 64 * slot + 0:64 * slot + 32] = WqT[:, h0 + o:h0 + o + 32]
            wqk[:, 0, 64 * slot + 32:64 * slot + 64] = WqT[:, h1 + o:h1 + o + 32]
            wqk[:, 1, 64 * slot + 0:64 * slot + 32] = WkT[:, h0 + o:h0 + o + 32]
            wqk[:, 1, 64 * slot + 32:64 * slot + 64] = WkT[:, h1 + o:h1 + o + 32]
            wqk[:, 2, 32 * slot:32 * slot + 32] = WqT[:, h2 + o:h2 + o + 32]
            wqk[:, 2, 64 + 32 * slot:64 + 32 * slot + 32] = WkT[:, h2 + o:h2 + o + 32]
            qkb[64 * slot + 0:64 * slot + 32, 0] = bq[h0 + o:h0 + o + 32]
            qkb[64 * slot + 32:64 * slot + 64, 0] = bq[h1 + o:h1 + o + 32]
            qkb[64 * slot + 0:64 * slot + 32, 1] = bk[h0 + o:h0 + o + 32]
            qkb[64 * slot + 32:64 * slot + 64, 1] = bk[h1 + o:h1 + o + 32]
            qkb[32 * slot:32 * slot + 32, 2] = bq[h2 + o:h2 + o + 32]
            qkb[64 + 32 * slot:64 + 32 * slot + 32, 2] = bk[h2 + o:h2 + o + 32]
        maps.append({
            "xT": np.ascontiguousarray(x[b].T).astype(BF),
            "wqk": wqk.astype(BF),
            "wv": np.ascontiguousarray(WvT[:, sl]).astype(BF),
            "woT": np.ascontiguousarray(Wo[:, sl].T).astype(BF),
            "qkb": qkb,
            "bvv": bv[sl].reshape(1, DH).astype(BF),
            "tmk": tri,
            "idm": np.eye(128, dtype=np.float32).astype(BF),
        })
    return maps


def kernel(x, Wq, bq, Wk, bk, Wv, bv, Wo, bo):
    if "nc" not in _CACHE:
        _CACHE["nc"] = build()
    nc = _CACHE["nc"]
    maps = _in_maps(x, Wq, bq, Wk, bk, Wv, bv, Wo, bo)
    res = run_bass_kernel_spmd(nc, maps, list(range(N_CORES))).results
    out = np.zeros((B, S, D), np.float32)
    for c in range(N_CORES):
        out[c // 4] += res[c]["outT"].astype(np.float32).T
    out += bo.astype(np.float32)
    return out
